# revision 1
# baseline (speedup 1.0000x reference)
"""Trainium2 Bass kernel for nn_DenseGraphWaveletLayer (v3).

out[:, l, :] = phi_l @ diag(theta) @ phi_inv_l @ (features[:, l, :] @ W)

Strategy (8 NeuronCores, SPMD single program, all-bf16 data path):
  - Nodes split into 128-row blocks, round-robin across cores; each core
    computes both sparse matmuls for its own output rows.
  - Per block: dma_gather source rows (128ch bf16 = 256B per edge; int16
    indices, so each 50k-row table is addressed in two halves with the
    block's edge stream split low/high by column), build scaled one-hot
    selection matrices on the vector engine (K-packed interleaved layout,
    2 ops per 8 tiles), accumulate on the tensor engine (one bf16 matmul
    per 128-edge tile).
  - spmm1 produces U^T in PSUM (W folded afterwards: z = U @ W); diag is
    folded into spmm1 edge values on the host.
  - z shards are AllGather'd (bf16); spmm2 repeats the machinery, writing
    fp32 output blocks.
  - 4 SWDGE queues parallelize gather descriptor generation.
"""

import os
import sys
import types

import numpy as np
import ml_dtypes

BF16 = ml_dtypes.bfloat16

N = 50000
L = 4
C = 128
NCORES = 8
BLK = 128
NB_TOT = (N + BLK - 1) // BLK            # 391
NBPC = (NB_TOT + NCORES - 1) // NCORES   # 49
SHARD = NBPC * BLK                       # 6272
TBL = NCORES * SHARD                     # 50176
HALF = 32768
GCAP = 1024                              # dma_gather num_idxs cap
KSEL = 8                                 # tiles per packed selection build
NQUEUES = 4


def _install_hook_stub():
    try:
        import antenv
    except ImportError:
        return
    try:
        from antenv import axon_hooks  # noqa: F401
        return
    except ImportError:
        pass
    mod = types.ModuleType("antenv.axon_hooks")
    mod._hook = None
    mod.set_axon_ntff_profile_hook = lambda h: setattr(mod, "_hook", h)
    mod.get_axon_ntff_profile_hook = lambda: mod._hook
    sys.modules["antenv.axon_hooks"] = mod
    antenv.axon_hooks = mod


def _table_index(cols):
    cblk = cols >> 7
    return (cblk & (NCORES - 1)) * SHARD + (cblk >> 3) * BLK + (cols & (BLK - 1))


def _edge_groups(rows):
    blk = rows >> 7
    return blk & (NCORES - 1), blk >> 3, rows & (BLK - 1)


def _count_halves(rows, ckey):
    core, k, _ = _edge_groups(rows)
    lo = ckey < HALF
    g = (core * NBPC + k) * 2 + (~lo)
    cnt = np.bincount(g, minlength=NCORES * NBPC * 2).reshape(NCORES, NBPC, 2)
    return cnt[..., 0], cnt[..., 1]


def _fill_slots(rows, ckey, vals, S_L, S_H):
    S = S_L + S_H
    core, k, rl = _edge_groups(rows)
    hi = (ckey >= HALF).astype(np.int8)
    idxv = np.where(hi == 0, ckey, ckey - HALF).astype(np.int16)
    order = np.lexsort((ckey, hi, k, core))
    core_s, k_s, hi_s = core[order], k[order], hi[order]
    g_s = (core_s * NBPC + k_s) * 2 + hi_s
    cnt = np.bincount(g_s, minlength=NCORES * NBPC * 2)
    starts = np.concatenate(([0], np.cumsum(cnt)[:-1]))
    pos = np.arange(len(order)) - starts[g_s]
    slot = pos + np.where(hi_s == 1, S_L, 0)
    idx_flat = np.zeros((NCORES, NBPC, S), np.int16)
    val = np.zeros((NCORES, NBPC, S), np.float32)
    rlf = np.zeros((NCORES, NBPC, S), np.float32)
    idx_flat[core_s, k_s, slot] = idxv[order]
    val[core_s, k_s, slot] = vals[order]
    rlf[core_s, k_s, slot] = rl[order]
    return idx_flat, val, rlf


def _wrap_idx(idx_flat):
    """[NC, NBPC, S] -> [NC, 128, NBPC*(S//16)]. Valid because all gather
    chunk boundaries (S_L and multiples of 1024 within each half) are
    16-aligned, so per-instruction wrapping equals global wrapping."""
    S = idx_flat.shape[-1]
    b16 = idx_flat.reshape(NCORES, NBPC, S // 16, 16).transpose(0, 1, 3, 2)
    b128 = np.tile(b16, (1, 1, 8, 1))
    return np.ascontiguousarray(
        b128.transpose(0, 2, 1, 3).reshape(NCORES, 128, NBPC * (S // 16)))


def _layout_aux(rlf, val):
    """2x [NC, NBPC, S] -> [NC, 128, NBPC*2*NT] bf16; per block cols
    [0:NT]=rl, [NT:2NT]=val, tile-major."""
    S = rlf.shape[-1]
    NT = S // 128

    def tm(x):
        return x.reshape(NCORES, NBPC, NT, 128).transpose(0, 1, 3, 2)

    a = np.concatenate([tm(rlf), tm(val)], axis=3)
    return np.ascontiguousarray(
        a.transpose(0, 2, 1, 3).reshape(NCORES, 128, NBPC * 2 * NT).astype(BF16))


def _preprocess(phi_indices, phi_values, phi_inverse_indices, phi_inverse_values,
                diagonal_weight_filter):
    rows1 = [phi_inverse_indices[l, 0].astype(np.int64) for l in range(L)]
    cols1 = [phi_inverse_indices[l, 1].astype(np.int64) for l in range(L)]
    rows2 = [phi_indices[l, 0].astype(np.int64) for l in range(L)]
    cols2 = [phi_indices[l, 1].astype(np.int64) for l in range(L)]

    max_lo = max_hi = 1
    for l in range(L):
        for rows, ckey in ((rows1[l], cols1[l]), (rows2[l], _table_index(cols2[l]))):
            nlo, nhi = _count_halves(rows, ckey)
            max_lo = max(max_lo, int(nlo.max()))
            max_hi = max(max_hi, int(nhi.max()))
    S_L = -(-max_lo // 128) * 128
    S_H = -(-max_hi // 128) * 128

    diag = np.asarray(diagonal_weight_filter, np.float64)
    idx1s, aux1s, idx2s, aux2s = [], [], [], []
    for l in range(L):
        v1 = (np.asarray(phi_inverse_values[l], np.float64) * diag[rows1[l]]
              ).astype(np.float32)
        i_f, val, rlf = _fill_slots(rows1[l], cols1[l], v1, S_L, S_H)
        idx1s.append(_wrap_idx(i_f))
        aux1s.append(_layout_aux(rlf, val))
        v2 = np.asarray(phi_values[l], np.float32)
        i_f, val, rlf = _fill_slots(rows2[l], _table_index(cols2[l]), v2, S_L, S_H)
        idx2s.append(_wrap_idx(i_f))
        aux2s.append(_layout_aux(rlf, val))

    idx1 = np.concatenate(idx1s, axis=2)
    aux1 = np.concatenate(aux1s, axis=2)
    idx2 = np.concatenate(idx2s, axis=2)
    aux2 = np.concatenate(aux2s, axis=2)
    return idx1, aux1, idx2, aux2, S_L, S_H


def _sel_groups(NT):
    out, g0 = [], 0
    while g0 < NT:
        out.append((g0, min(KSEL, NT - g0)))
        g0 += KSEL
    return out


def _gchunks(S_L, S_H, cap):
    for g0 in range(0, S_L, cap):
        yield g0, min(cap, S_L - g0), 0
    for g0 in range(0, S_H, cap):
        yield S_L + g0, min(cap, S_H - g0), 1


def _build(S_L, S_H, scales, nblocks):
    import concourse.mybir as mybir
    import concourse.tile as tile
    from concourse import bacc

    S = S_L + S_H
    NT = S // 128
    S16 = S // 16
    f32 = mybir.dt.float32
    bf16 = mybir.dt.bfloat16
    i16 = mybir.dt.int16
    eq = mybir.AluOpType.is_equal
    mult = mybir.AluOpType.mult
    AF = mybir.ActivationFunctionType
    groups = _sel_groups(NT)
    ksizes = sorted({k for _, k in groups})

    nc = bacc.Bacc("TRN2", target_bir_lowering=False, debug=False,
                   num_devices=NCORES, num_swdge_queues=NQUEUES)
    featsB = nc.dram_tensor("featsB", [L * N, C], bf16, kind="ExternalInput")
    wmat = nc.dram_tensor("wmat", [C, C], bf16, kind="ExternalInput")
    iotas = {k: nc.dram_tensor(f"iota{k}", [128, k * 128], bf16,
                               kind="ExternalInput") for k in ksizes}
    idx1 = nc.dram_tensor("idx1", [128, L * NBPC * S16], i16, kind="ExternalInput")
    idx2 = nc.dram_tensor("idx2", [128, L * NBPC * S16], i16, kind="ExternalInput")
    aux1 = nc.dram_tensor("aux1", [128, L * NBPC * 2 * NT], bf16, kind="ExternalInput")
    aux2 = nc.dram_tensor("aux2", [128, L * NBPC * 2 * NT], bf16, kind="ExternalInput")
    outp = nc.dram_tensor("outp", [L, SHARD, C], f32, kind="ExternalOutput")

    qn = [0]

    with tile.TileContext(nc) as tc:
        with (
            tc.tile_pool(name="const", bufs=1) as constp,
            tc.tile_pool(name="aux", bufs=8) as auxp,
            tc.tile_pool(name="dst", bufs=4) as dstp,
            tc.tile_pool(name="sel", bufs=8) as selp,
            tc.tile_pool(name="stg", bufs=6) as stgp,
            tc.tile_pool(name="psU", bufs=3, space="PSUM") as psUp,
            tc.tile_pool(name="psZ", bufs=2, space="PSUM") as psZp,
            tc.tile_pool(name="psO", bufs=3, space="PSUM") as psOp,
            tc.tile_pool(name="dram", bufs=4, space="DRAM") as dramp,
        ):
            w_t = constp.tile([C, C], bf16)
            nc.sync.dma_start(w_t[:], wmat[:])
            io_t = {}
            for k in ksizes:
                io_t[k] = constp.tile([128, k * 128], bf16, tag=f"iota{k}",
                                      name=f"io_t{k}")
                nc.sync.dma_start(io_t[k][:], iotas[k][:])

            def gathers(dst, it, src_lo, src_hi):
                for r, n, h in _gchunks(S_L, S_H, GCAP):
                    nc.gpsimd.dma_gather(
                        dst[:, r // 128:(r + n) // 128, :],
                        src_hi if h else src_lo,
                        it[:, r // 16:(r + n) // 16], n, n, C,
                        queue_num=qn[0] % NQUEUES)
                    qn[0] += 1

            def build_sel(vt, g0, K):
                """Interleaved [p, r*K + k] scaled one-hot for tiles
                g0..g0+K; tile k slice = sel[:, k::K]."""
                rl_b = vt[:, g0:g0 + K].to_broadcast(
                    [128, K, 128]).rearrange("p k r -> p r k")
                v_b = vt[:, NT + g0:NT + g0 + K].to_broadcast(
                    [128, K, 128]).rearrange("p k r -> p r k")
                sel = selp.tile([128, K * 128], bf16, tag=f"sel{K}",
                                name=f"sel{K}")
                s3 = sel[:].rearrange("p (r k) -> p r k", k=K)
                io_v = io_t[K][:].rearrange("p (r k) -> p r k", k=K)
                nc.vector.tensor_tensor(out=s3, in0=rl_b, in1=io_v, op=eq)
                nc.vector.tensor_tensor(out=s3, in0=s3, in1=v_b, op=mult)
                return sel

            ztbs = []
            for l in range(scales):
                zsh = dramp.tile([SHARD, C], bf16, tag="zsh")
                ztb = dramp.tile([TBL, C], bf16, tag="ztb", addr_space="Shared")
                ztbs.append(ztb)
                # ---- spmm1: psU[ci, r] += g.T @ sel ; z = U @ W ----
                for b in range(nblocks):
                    cb = l * NBPC + b
                    it = auxp.tile([128, S16], i16, tag="idx")
                    nc.sync.dma_start(it[:], idx1[:, cb * S16:(cb + 1) * S16])
                    vt = auxp.tile([128, 2 * NT], bf16, tag="vr")
                    nc.sync.dma_start(vt[:], aux1[:, cb * 2 * NT:(cb + 1) * 2 * NT])
                    dst = dstp.tile([128, NT, C], bf16, tag="dst")
                    gathers(dst, it, featsB[l * N:l * N + HALF, :],
                            featsB[l * N + HALF:(l + 1) * N, :])
                    psU = psUp.tile([128, 128], f32)
                    for g0, K in groups:
                        sel = build_sel(vt, g0, K)
                        for k in range(K):
                            t = g0 + k
                            nc.tensor.matmul(
                                psU[:], lhsT=dst[:, t, :], rhs=sel[:, k::K],
                                start=(t == 0), stop=(t == NT - 1))
                    ut = stgp.tile([128, 128], bf16, tag="ut")
                    nc.scalar.activation(ut[:], psU[:], AF.Copy)
                    psZ = psZp.tile([128, 128], f32)
                    nc.tensor.matmul(psZ[:], lhsT=ut[:], rhs=w_t[:],
                                     start=True, stop=True)
                    zt = stgp.tile([128, 128], bf16, tag="zt")
                    nc.scalar.activation(zt[:], psZ[:], AF.Copy)
                    nc.sync.dma_start(zsh[b * BLK:(b + 1) * BLK, :], zt[:])

                nc.gpsimd.collective_compute(
                    "AllGather", mybir.AluOpType.bypass,
                    replica_groups=[list(range(NCORES))],
                    ins=[zsh.opt()], outs=[ztb.opt()])

            # ---- spmm2: psO[r, co] += sel.T @ g ----
            for l in range(scales):
                ztb = ztbs[l]
                for b in range(nblocks):
                    cb = l * NBPC + b
                    it = auxp.tile([128, S16], i16, tag="idx")
                    nc.sync.dma_start(it[:], idx2[:, cb * S16:(cb + 1) * S16])
                    vt = auxp.tile([128, 2 * NT], bf16, tag="vr")
                    nc.sync.dma_start(vt[:], aux2[:, cb * 2 * NT:(cb + 1) * 2 * NT])
                    dst = dstp.tile([128, NT, C], bf16, tag="dst")
                    gathers(dst, it, ztb[0:HALF, :], ztb[HALF:TBL, :])
                    psO = psOp.tile([128, 128], f32)
                    for g0, K in groups:
                        sel = build_sel(vt, g0, K)
                        for k in range(K):
                            t = g0 + k
                            nc.tensor.matmul(
                                psO[:], lhsT=sel[:, k::K], rhs=dst[:, t, :],
                                start=(t == 0), stop=(t == NT - 1))
                    ot = stgp.tile([128, 128], f32, tag="ot")
                    nc.vector.tensor_copy(ot[:], psO[:])
                    nc.sync.dma_start(outp[l, b * BLK:(b + 1) * BLK, :], ot[:])
    nc.compile()
    return nc


def kernel(**inputs):
    _install_hook_stub()
    from concourse.bass_utils import run_bass_kernel_spmd

    feats = np.asarray(inputs["features"], np.float32)        # [N, L, C]
    featsB = np.ascontiguousarray(
        feats.transpose(1, 0, 2).reshape(L * N, C)).astype(BF16)
    wmat = np.asarray(inputs["weight_matrix"], np.float32).astype(BF16)

    idx1, aux1, idx2, aux2, S_L, S_H = _preprocess(
        np.asarray(inputs["phi_indices"]), np.asarray(inputs["phi_values"]),
        np.asarray(inputs["phi_inverse_indices"]),
        np.asarray(inputs["phi_inverse_values"]),
        np.asarray(inputs["diagonal_weight_filter"]))
    NT = (S_L + S_H) // 128

    scales = int(os.environ.get("DGW_SCALES", L))
    nblocks = int(os.environ.get("DGW_BLOCKS", NBPC))
    nc = _build(S_L, S_H, scales, nblocks)

    iotas = {}
    for _, K in _sel_groups(NT):
        if K not in iotas:
            iotas[K] = np.ascontiguousarray(np.tile(
                np.arange(128, dtype=np.float32)[None, :, None],
                (128, 1, K)).reshape(128, K * 128)).astype(BF16)

    in_maps = []
    for c in range(NCORES):
        m = dict(featsB=featsB, wmat=wmat,
                 idx1=np.ascontiguousarray(idx1[c]),
                 idx2=np.ascontiguousarray(idx2[c]),
                 aux1=np.ascontiguousarray(aux1[c]),
                 aux2=np.ascontiguousarray(aux2[c]))
        for K, arr in iotas.items():
            m[f"iota{K}"] = arr
        in_maps.append(m)
    res = run_bass_kernel_spmd(nc, in_maps, core_ids=list(range(NCORES)))
    kernel.last_results = res

    shards = np.stack([res.results[c]["outp"] for c in range(NCORES)])
    blocks = shards.reshape(NCORES, L, NBPC, BLK, C).transpose(2, 0, 3, 1, 4)
    out = blocks.reshape(NBPC * NCORES * BLK, L, C)[:N]
    return np.ascontiguousarray(out)

